# revision 11
# baseline (speedup 1.0000x reference)
import numpy as np

B = 8
SEQ = 4096
D = 1024
N_BASE = 10000.0
N_CORES = 8
SPC = SEQ // N_CORES   # seq rows per core (512)
H = 128                # f32 per 512B unit
UPP = 32               # units per partition per chunk (16KB)
UPC = SPC * D // H     # units per core chunk (4096)

_CACHE = {}


def _compute_pe() -> np.ndarray:
    """Mirror of the reference _pos_encoding (default jax backend, f32)."""
    import jax
    import jax.numpy as jnp

    pos = jnp.arange(SEQ, dtype=jnp.float32)[:, None]
    i = jnp.arange(D // 2, dtype=jnp.float32)
    denom = jnp.power(jnp.float32(N_BASE), 2.0 * i / jnp.float32(D))
    ang = pos / denom
    pe = jnp.stack([jnp.sin(ang), jnp.cos(ang)], axis=-1).reshape(SEQ, D)
    return np.asarray(jax.device_get(pe), dtype=np.float32)


def _pass_dmas(nc, engine, dram, row0, tile, u0, nu, to_sbuf, skip15):
    """Move [128 parts x nu units] between the chunk at dram[row0:]
    (natural order: partition p holds units [UPP*p, UPP*p+UPP)) and
    tile[:, u0:u0+nu, :], for the unit column range [u0, u0+nu).

    skip15=False: one [128]-DMA -> 16 engines x 8 descs (uniform).
    skip15=True: a [120]-DMA (engines 0-14) + an [8]-DMA (engines 0-7),
    so SDMA engine 15 (~17% slower than 0-14) gets nothing. HWDGE splits
    a DMA's n descriptors into runs of g = smallest divisor of n that is
    >= n/16, assigned to engines 0..n/g-1.
    """
    view = dram[row0 : row0 + 128 * UPP, :].rearrange(
        "(p j) d -> p j d", j=UPP
    )
    us = slice(u0, u0 + nu)
    if not skip15:
        pairs = [(tile[:, us, :], view[:, us, :])]
    else:
        pairs = [
            (tile[0:120, us, :], view[0:120, us, :]),
            (tile[120:128, us, :], view[120:128, us, :]),
        ]
    for sb, dr in pairs:
        if to_sbuf:
            engine.dma_start(out=sb, in_=dr)
        else:
            engine.dma_start(out=dr, in_=sb)


def _build_program():
    import concourse.bacc as bacc
    import concourse.mybir as mybir
    import concourse.tile as tile

    nc = bacc.Bacc("TRN2")
    f32 = mybir.dt.float32
    x_in = nc.declare_dram_parameter("x", [B * UPC, H], f32, isOutput=False)
    pe_in = nc.declare_dram_parameter("pe", [UPC, H], f32, isOutput=False)
    y_out = nc.declare_dram_parameter("y", [B * UPC, H], f32, isOutput=True)

    with tile.TileContext(nc) as tc:
        with (
            tc.tile_pool(name="pe_pool", bufs=1) as pe_pool,
            tc.tile_pool(name="x_pool", bufs=B) as x_pool,
        ):
            pe_t = pe_pool.tile([128, UPP, H], f32)
            # pe rides the (initially idle) scalar/output queue, skipping
            # engine 15 so the slow engine only carries uniform passes.
            _pass_dmas(nc, nc.scalar, pe_in, 0, pe_t, 0, UPP, True, True)
            xts = []
            for b in range(B):
                xt = x_pool.tile([128, UPP, H], f32)
                if b < B - 1:
                    _pass_dmas(
                        nc, nc.sync, x_in, b * UPC, xt, 0, UPP, True, False
                    )
                else:
                    # last chunk: two half-passes so the final add is
                    # short; first half engine-15-free for phase balance
                    hu = UPP // 2
                    _pass_dmas(
                        nc, nc.sync, x_in, b * UPC, xt, 0, hu, True, True
                    )
                    _pass_dmas(
                        nc, nc.sync, x_in, b * UPC, xt, hu, hu, True, False
                    )
                xts.append(xt)
            hu = UPP // 2
            for b in range(B - 1):
                nc.vector.tensor_add(xts[b][:], xts[b][:], pe_t[:])
            for hi in range(2):
                sl = slice(hi * hu, (hi + 1) * hu)
                b = B - 1
                nc.vector.tensor_add(
                    xts[b][:, sl, :], xts[b][:, sl, :], pe_t[:, sl, :]
                )
            # y DMAs issued in REVERSE chunk order: the out-ring head
            # blocks on the last chunk's add, so the output stream only
            # starts once the whole input stream has drained — each
            # direction then runs alone at full per-engine rate (mixed
            # in/out ring interleaving costs ~11% per descriptor).
            for hi in (1, 0):
                sl = slice(hi * hu, (hi + 1) * hu)
                b = B - 1
                _pass_dmas(
                    nc,
                    nc.scalar,
                    y_out,
                    b * UPC,
                    xts[b],
                    hi * hu,
                    hu,
                    False,
                    True,
                )
            for b in range(B - 2, -1, -1):
                _pass_dmas(
                    nc, nc.scalar, y_out, b * UPC, xts[b], 0, UPP, False, False
                )
    if not nc.is_finalized():
        nc.finalize()
    return nc


def _get_state():
    if "nc" not in _CACHE:
        _CACHE["nc"] = _build_program()
    if "pe" not in _CACHE:
        _CACHE["pe"] = _compute_pe()
    return _CACHE["nc"], _CACHE["pe"]


def _in_maps(x, pe):
    in_maps = []
    for c in range(N_CORES):
        xs = np.ascontiguousarray(x[:, c * SPC : (c + 1) * SPC, :]).reshape(
            B * UPC, H
        )
        pes = np.ascontiguousarray(pe[c * SPC : (c + 1) * SPC, :]).reshape(
            UPC, H
        )
        in_maps.append({"x": xs, "pe": pes})
    return in_maps


def kernel(x, seq_len=None, **_):
    from concourse.bass_utils import run_bass_kernel_spmd

    x = np.asarray(x, dtype=np.float32)
    assert x.shape == (B, SEQ, D)
    if seq_len is not None:
        assert int(np.asarray(seq_len)) == SEQ

    nc, pe = _get_state()
    res = run_bass_kernel_spmd(nc, _in_maps(x, pe), list(range(N_CORES))).results

    out = np.empty((B, SEQ, D), dtype=np.float32)
    for c in range(N_CORES):
        out[:, c * SPC : (c + 1) * SPC, :] = res[c]["y"].reshape(B, SPC, D)
    return out


# revision 12
# speedup vs baseline: 1.0955x; 1.0955x over previous
import numpy as np

B = 8
SEQ = 4096
D = 1024
N_BASE = 10000.0
N_CORES = 8
SPC = SEQ // N_CORES   # seq rows per core (512)
H = 128                # f32 per 512B unit
UPP = 32               # units per partition per chunk (16KB)
UPC = SPC * D // H     # units per core chunk (4096)

_CACHE = {}


def _compute_pe() -> np.ndarray:
    """Mirror of the reference _pos_encoding (default jax backend, f32)."""
    import jax
    import jax.numpy as jnp

    pos = jnp.arange(SEQ, dtype=jnp.float32)[:, None]
    i = jnp.arange(D // 2, dtype=jnp.float32)
    denom = jnp.power(jnp.float32(N_BASE), 2.0 * i / jnp.float32(D))
    ang = pos / denom
    pe = jnp.stack([jnp.sin(ang), jnp.cos(ang)], axis=-1).reshape(SEQ, D)
    return np.asarray(jax.device_get(pe), dtype=np.float32)


def _pass_dmas(nc, engine, dram, row0, tile, u0, nu, to_sbuf, skip15):
    """Move [128 parts x nu units] between the chunk at dram[row0:]
    (natural order: partition p holds units [UPP*p, UPP*p+UPP)) and
    tile[:, u0:u0+nu, :], for the unit column range [u0, u0+nu).

    skip15=False: one [128]-DMA -> 16 engines x nu/4 descs (uniform).
    skip15=True: a [120]-DMA (engines 0-14) + an [8]-DMA (engines 0-7),
    so SDMA engine 15 (~17% slower than 0-14) gets nothing. HWDGE splits
    a DMA's n descriptors into runs of g = smallest divisor of n that is
    >= n/16, assigned to engines 0..n/g-1.
    """
    view = dram[row0 : row0 + 128 * UPP, :].rearrange(
        "(p j) d -> p j d", j=UPP
    )
    us = slice(u0, u0 + nu)
    if not skip15:
        pairs = [(tile[:, us, :], view[:, us, :])]
    else:
        pairs = [
            (tile[0:120, us, :], view[0:120, us, :]),
            (tile[120:128, us, :], view[120:128, us, :]),
        ]
    for sb, dr in pairs:
        if to_sbuf:
            engine.dma_start(out=sb, in_=dr)
        else:
            engine.dma_start(out=dr, in_=sb)


# per-chunk sub-pass plans: (u0, nu, x_skip15, y_skip15)
# chunk 0 ramps in small so the first add/write starts early; chunk 7
# ramps out small so the final add barely delays the last writes.
_PLANS = {
    0: [(0, 8, False, False), (8, 8, False, False), (16, 16, False, False)],
    B - 1: [(0, 16, True, True), (16, 8, False, True), (24, 8, False, True)],
}
_FULL = [(0, UPP, False, False)]


def _build_program():
    import concourse.bacc as bacc
    import concourse.mybir as mybir
    import concourse.tile as tile

    nc = bacc.Bacc("TRN2")
    f32 = mybir.dt.float32
    x_in = nc.declare_dram_parameter("x", [B * UPC, H], f32, isOutput=False)
    pe_in = nc.declare_dram_parameter("pe", [UPC, H], f32, isOutput=False)
    y_out = nc.declare_dram_parameter("y", [B * UPC, H], f32, isOutput=True)

    with tile.TileContext(nc) as tc:
        with (
            tc.tile_pool(name="pe_pool", bufs=1) as pe_pool,
            tc.tile_pool(name="x_pool", bufs=B) as x_pool,
        ):
            pe_t = pe_pool.tile([128, UPP, H], f32)
            # pe halves ride both rings up front (engine-15-free); adds
            # only depend on the pe columns they touch, so the pipeline
            # starts as soon as the first half lands.
            _pass_dmas(nc, nc.sync, pe_in, 0, pe_t, 0, 16, True, True)
            _pass_dmas(nc, nc.scalar, pe_in, 0, pe_t, 16, 16, True, True)
            xts = []
            for b in range(B):
                xt = x_pool.tile([128, UPP, H], f32)
                for u0, nu, xs15, _ys15 in _PLANS.get(b, _FULL):
                    _pass_dmas(
                        nc, nc.sync, x_in, b * UPC, xt, u0, nu, True, xs15
                    )
                xts.append(xt)
            for b in range(B):
                for u0, nu, _xs15, ys15 in _PLANS.get(b, _FULL):
                    sl = slice(u0, u0 + nu)
                    nc.vector.tensor_add(
                        xts[b][:, sl, :], xts[b][:, sl, :], pe_t[:, sl, :]
                    )
                    _pass_dmas(
                        nc, nc.scalar, y_out, b * UPC, xts[b], u0, nu,
                        False, ys15,
                    )
    if not nc.is_finalized():
        nc.finalize()
    return nc


def _get_state():
    if "nc" not in _CACHE:
        _CACHE["nc"] = _build_program()
    if "pe" not in _CACHE:
        _CACHE["pe"] = _compute_pe()
    return _CACHE["nc"], _CACHE["pe"]


def _in_maps(x, pe):
    in_maps = []
    for c in range(N_CORES):
        xs = np.ascontiguousarray(x[:, c * SPC : (c + 1) * SPC, :]).reshape(
            B * UPC, H
        )
        pes = np.ascontiguousarray(pe[c * SPC : (c + 1) * SPC, :]).reshape(
            UPC, H
        )
        in_maps.append({"x": xs, "pe": pes})
    return in_maps


def kernel(x, seq_len=None, **_):
    from concourse.bass_utils import run_bass_kernel_spmd

    x = np.asarray(x, dtype=np.float32)
    assert x.shape == (B, SEQ, D)
    if seq_len is not None:
        assert int(np.asarray(seq_len)) == SEQ

    nc, pe = _get_state()
    res = run_bass_kernel_spmd(nc, _in_maps(x, pe), list(range(N_CORES))).results

    out = np.empty((B, SEQ, D), dtype=np.float32)
    for c in range(N_CORES):
        out[:, c * SPC : (c + 1) * SPC, :] = res[c]["y"].reshape(B, SPC, D)
    return out
